# revision 18
# baseline (speedup 1.0000x reference)
"""Trainium2 Bass kernel for nn_AutomatonPT (3D cellular automaton / GNN message passing).

Full inputs -> full output. Shards the X axis across 8 NeuronCores (6 planes
each + 1 halo plane per side, periodic). Uses the antisymmetry of the pair
flux (F_ji = -F_ij) to evaluate only 13 of the 26 stencil shifts.

MLP chain in bf16 with fp32 PSUM accumulation; ACT reads PSUM; 4 independent
chains interleaved at chunk granularity with layer-0 tanh software-pipelined
one group ahead; epilogue in bf16 with exact +-1 scatter coefficients (the
dinv*SCALE factors are folded into the charge tensors); final add in fp32.
"""
import sys

sys.path.insert(0, "/opt/trn_rl_repo")
from contextlib import ExitStack

import numpy as np

import concourse.bass as bass
import concourse.bacc as bacc
import concourse.tile as tile
from concourse import mybir
from concourse.bass_utils import run_bass_kernel_spmd

F32 = mybir.dt.float32
BF16 = mybir.dt.bfloat16
ALU = mybir.AluOpType
ACTF = mybir.ActivationFunctionType

N_CORES = 8
NX = 48
PLANES = 8
OWN = 6
YZ = 48 * 48
PAD = 50 * 50

SCALE = 0.05234482976098482 * 0.8
S2 = 2 ** -0.5
S3 = 3 ** -0.5
# unordered pair deltas d = j - i: 9 with dx=1, then 4 with dx=0
SHIFTS_U = [
    (1, 0, 0, 1.0),
    (1, 1, 0, S2), (1, -1, 0, S2), (1, 0, 1, S2), (1, 0, -1, S2),
    (1, 1, 1, S3), (1, 1, -1, S3), (1, -1, 1, S3), (1, -1, -1, S3),
    (0, 1, 0, 1.0), (0, 0, 1, 1.0),
    (0, 1, 1, S2), (0, 1, -1, S2),
]

H_CHUNKS = [(0, 512), (512, 512), (1024, 512), (1536, 512), (2048, 256)]
MM_N = 512
PSF_CHUNKS = [(0, 512), (512, 512), (1024, 512), (1536, 512), (2048, 256)]
ROW_CHUNKS = [(0, 10), (10, 10), (20, 10), (30, 10), (40, 8)]
GROUP = 2


def _v3(ap):
    return ap.rearrange("p (y z) -> p y z", y=48)


def device_kernel(tc, reps=1):
    nc = tc.nc
    t = {}
    t["qpad"] = nc.dram_tensor("qpad", [PLANES, 2, 50, 50], F32, kind="ExternalInput")
    for n in ("lhtA", "lhtB", "lhtAs", "lhtBs"):
        t[n] = nc.dram_tensor(n, [16, 128], BF16, kind="ExternalInput")
    for n in ("lht1", "lht2", "lht3"):
        t[n] = nc.dram_tensor(n, [128, 128], BF16, kind="ExternalInput")
    t["lhtOp"] = nc.dram_tensor("lhtOp", [128, 8], BF16, kind="ExternalInput")
    t["lhtOm"] = nc.dram_tensor("lhtOm", [128, 8], BF16, kind="ExternalInput")
    for n in ("b0v", "b1v", "b2v", "b3v"):
        t[n] = nc.dram_tensor(n, [128, 1], F32, kind="ExternalInput")
    t["lhtSp"] = nc.dram_tensor("lhtSp", [128, 8], BF16, kind="ExternalInput")
    t["lhtSm"] = nc.dram_tensor("lhtSm", [128, 8], BF16, kind="ExternalInput")
    t["cvec"] = nc.dram_tensor("cvec", [128, 1], F32, kind="ExternalInput")
    t["out0"] = nc.dram_tensor("out0", [OWN, 48, 48], F32, kind="ExternalOutput")

    with ExitStack() as ctx:
        persist = ctx.enter_context(tc.tile_pool(name="persist", bufs=1))
        mmps = ctx.enter_context(tc.tile_pool(name="mmps", bufs=6, space="PSUM"))
        psf = ctx.enter_context(tc.tile_pool(name="psf", bufs=2, space="PSUM"))

        w = {}
        wspecs = [("lhtA", [16, 128], BF16), ("lhtB", [16, 128], BF16),
                  ("lhtAs", [16, 128], BF16), ("lhtBs", [16, 128], BF16),
                  ("lht1", [128, 128], BF16), ("lht2", [128, 128], BF16),
                  ("lht3", [128, 128], BF16), ("lhtOp", [128, 8], BF16),
                  ("lhtOm", [128, 8], BF16), ("b0v", [128, 1], F32),
                  ("b1v", [128, 1], F32), ("b2v", [128, 1], F32),
                  ("b3v", [128, 1], F32), ("lhtSp", [128, 8], BF16),
                  ("lhtSm", [128, 8], BF16), ("cvec", [128, 1], F32)]
        for n, shape, dt in wspecs:
            w[n] = persist.tile(shape, dt, tag=n, name=n)
            nc.sync.dma_start(out=w[n], in_=t[n][:])

        fstack = persist.tile([128, YZ], BF16, tag="fstack")
        nc.vector.memset(fstack[96:128, :], 0.0)
        qc8 = persist.tile([8, 50, 50], F32, tag="qc8")
        nc.sync.dma_start(out=qc8, in_=t["qpad"][:, 0])
        # charge tensors in bf16, pre-scaled by dinv*SCALE (cvec), built early
        qc8b = persist.tile([8, 50, 50], BF16, tag="qc8b")
        nc.vector.tensor_copy(out=qc8b, in_=qc8)
        qcs8b = persist.tile([8, 50, 50], BF16, tag="qcs8b")
        nc.vector.memset(qcs8b[0:8], 0.0)
        nc.sync.dma_start(out=qcs8b[0:7], in_=qc8b[1:8])
        qo_rep = persist.tile([128, YZ], BF16, tag="qo")
        qn_rep = persist.tile([128, YZ], BF16, tag="qn")
        nc.vector.memset(qo_rep[96:128, :], 0.0)
        nc.vector.memset(qn_rep[96:128, :], 0.0)
        qo3, qn3 = _v3(qo_rep), _v3(qn_rep)
        for s, (dx, dy, dz, _) in enumerate(SHIFTS_U):
            ay, az = 1 + dy, 1 + dz
            nc.sync.dma_start(out=qo3[8 * s:8 * s + 8], in_=qc8b[:, 1:49, 1:49])
            qsrc = qcs8b if dx == 1 else qc8b
            nc.sync.dma_start(out=qn3[8 * s:8 * s + 8],
                              in_=qsrc[:, ay:ay + 48, az:az + 48])
        nc.vector.tensor_scalar_mul(out=qo_rep, in0=qo_rep, scalar1=w["cvec"])
        nc.vector.tensor_scalar_mul(out=qn_rep, in0=qn_rep, scalar1=w["cvec"])

        for _rep in range(reps):
          with tc.tile_pool(name=f"abfam{_rep}", bufs=1) as abfam:
            A8pad = abfam.tile([128, 50, 50], BF16, tag="A8pad")
            B8pad = abfam.tile([128, 50, 50], BF16, tag="B8pad")
            A8s = abfam.tile([128, 50, 50], BF16, tag="A8s")
            B8s = abfam.tile([128, 50, 50], BF16, tag="B8s")

            # ---- phase A: layer-0 features (fp32 matmul -> bf16) ----
            with tc.tile_pool(name=f"qpool{_rep}", bufs=1) as qpool:
                q16 = qpool.tile([16, PAD], F32, tag="q16")
                nc.sync.dma_start(out=q16,
                                  in_=t["qpad"][:].rearrange("p c y z -> (p c) (y z)"))
                q16b = qpool.tile([16, PAD], BF16, tag="q16b")
                nc.vector.tensor_copy(out=q16b, in_=q16)
                for dst, lht in ((A8pad, "lhtA"), (B8pad, "lhtB"),
                                 (A8s, "lhtAs"), (B8s, "lhtBs")):
                    dflat = dst.rearrange("p y z -> p (y z)")
                    for off in range(0, PAD, MM_N):
                        n = min(MM_N, PAD - off)
                        ps = mmps.tile([128, n], F32, tag="mm", name="mm")
                        nc.tensor.matmul(ps, w[lht], q16b[:, off:off + n],
                                         start=True, stop=True)
                        nc.scalar.copy(out=dflat[:, off:off + n], in_=ps)

            # ---- phase B: shifts x 2 directions, interleaved chains ----
            with tc.tile_pool(name=f"pre{_rep}", bufs=12) as prep, \
                 tc.tile_pool(name=f"hp{_rep}", bufs=12) as hp, \
                 tc.tile_pool(name=f"h3p{_rep}", bufs=3) as h3p, \
                 tc.tile_pool(name=f"fsp{_rep}", bufs=2) as fsp:

                def emit_pre(s):
                    dx, dy, dz, _ = SHIFTS_U[s]
                    f1pre = prep.tile([128, YZ], BF16, tag="pre", name="pre")
                    f2pre = prep.tile([128, YZ], BF16, tag="pre", name="pre")
                    ay, az = 1 + dy, 1 + dz
                    if dx == 1:
                        nc.vector.tensor_add(out=_v3(f1pre),
                                             in0=A8pad[:, 1:49, 1:49],
                                             in1=B8s[:, ay:ay + 48, az:az + 48])
                        nc.vector.tensor_add(out=_v3(f2pre),
                                             in0=A8s[:, ay:ay + 48, az:az + 48],
                                             in1=B8pad[:, 1:49, 1:49])
                    else:
                        nc.vector.tensor_add(out=_v3(f1pre),
                                             in0=A8pad[:, 1:49, 1:49],
                                             in1=B8pad[:, ay:ay + 48, az:az + 48])
                        nc.vector.tensor_add(out=_v3(f2pre),
                                             in0=A8pad[:, ay:ay + 48, az:az + 48],
                                             in1=B8pad[:, 1:49, 1:49])
                    return [f1pre, f2pre]

                def emit_h0s(pres):
                    h0s = [hp.tile([128, YZ], BF16, tag="h", name="h")
                           for _ in pres]
                    for h0, pre in zip(h0s, pres):
                        nc.scalar.activation(out=h0, in_=pre, func=ACTF.Tanh,
                                             bias=w["b0v"], scale=1.0)
                    return h0s

                LAYERS = [("lht1", "b1v"), ("lht2", "b2v"), ("lht3", "b3v")]

                def emit_group(shifts, chains):
                    for li, (lht, bv) in enumerate(LAYERS):
                        nxt = []
                        for ci in range(len(chains)):
                            if li == 2:
                                kt = "h3a" if ci % 2 == 0 else "h3b"
                                nxt.append(h3p.tile([128, YZ], BF16, tag=kt, name=kt))
                            else:
                                nxt.append(hp.tile([128, YZ], BF16, tag="h", name="h"))
                        for off, csz in H_CHUNKS:
                            pss = []
                            for ci, hcur in enumerate(chains):
                                ps = mmps.tile([128, csz], F32, tag="mm", name="mm")
                                for o2 in range(0, csz, MM_N):
                                    n2 = min(MM_N, csz - o2)
                                    nc.tensor.matmul(ps[:, o2:o2 + n2], w[lht],
                                                     hcur[:, off + o2:off + o2 + n2],
                                                     start=True, stop=True)
                                pss.append(ps)
                            for ci, ps in enumerate(pss):
                                nc.scalar.activation(out=nxt[ci][:, off:off + csz],
                                                     in_=ps, func=ACTF.Tanh,
                                                     bias=w[bv], scale=1.0)
                        chains = nxt
                    # tail: fdiff -> tanh -> fstack, interleaved across shifts
                    fss = {s: fsp.tile([8, YZ], BF16, tag="fs", name="fs")
                           for s in shifts}
                    for off, csz in PSF_CHUNKS:
                        pfs = {}
                        for gi, s in enumerate(shifts):
                            h3f1, h3f2 = chains[2 * gi], chains[2 * gi + 1]
                            pf = psf.tile([8, csz], F32, tag="psf", name="psf")
                            nc.tensor.matmul(pf, w["lhtOp"], h3f1[:, off:off + csz],
                                             start=True, stop=False)
                            nc.tensor.matmul(pf, w["lhtOm"], h3f2[:, off:off + csz],
                                             start=False, stop=True)
                            pfs[s] = pf
                        for s in shifts:
                            nc.scalar.activation(out=fss[s][:, off:off + csz],
                                                 in_=pfs[s], func=ACTF.Tanh)
                    for s in shifts:
                        nc.sync.dma_start(out=fstack[8 * s:8 * s + 8, :], in_=fss[s])

                groups = [list(range(i, min(i + GROUP, 13)))
                          for i in range(0, 13, GROUP)]
                # software pipeline: pre + h0 of group g+1 emitted before layers(g)
                h0bank = emit_h0s([p for s in groups[0] for p in emit_pre(s)])
                for g, shifts in enumerate(groups):
                    if g + 1 < len(groups):
                        h0_next = emit_h0s(
                            [p for s in groups[g + 1] for p in emit_pre(s)])
                    else:
                        h0_next = None
                    emit_group(shifts, h0bank)
                    h0bank = h0_next

          # ---- phase C: epilogue ----
          with tc.tile_pool(name=f"epi{_rep}", bufs=1) as epi:
            qco = epi.tile([6, YZ], F32, tag="qco")
            nc.sync.dma_start(out=_v3(qco), in_=qc8[1:7, 1:49, 1:49])
            Fq = epi.tile([128, YZ], BF16, tag="Fq")
            Fpad = epi.tile([128, 50, 50], BF16, tag="Fpad")
            nc.vector.scalar_tensor_tensor(out=Fq, in0=fstack, scalar=0.0,
                                           in1=qo_rep, op0=ALU.min, op1=ALU.mult)
            nc.vector.scalar_tensor_tensor(out=Fpad[:, 1:49, 1:49], in0=_v3(fstack),
                                           scalar=0.0, in1=qn3,
                                           op0=ALU.max, op1=ALU.mult)
            nc.vector.tensor_add(out=Fpad[:, 1:49, 1:49], in0=Fpad[:, 1:49, 1:49],
                                 in1=_v3(Fq))
            nc.sync.dma_start(out=Fpad[:, 1:49, 0:1], in_=Fpad[:, 1:49, 48:49])
            nc.sync.dma_start(out=Fpad[:, 1:49, 49:50], in_=Fpad[:, 1:49, 1:2])
            nc.sync.dma_start(out=Fpad[:, 0:1, 0:50], in_=Fpad[:, 48:49, 0:50])
            nc.sync.dma_start(out=Fpad[:, 49:50, 0:50], in_=Fpad[:, 1:2, 0:50])

            # pre-rolled F stack for the single merged minus-scatter matmul
            Fm = epi.tile([128, YZ], BF16, tag="Fm")
            Fm3 = _v3(Fm)
            for s, (dx, dy, dz, _) in enumerate(SHIFTS_U):
                my, mz = 1 - dy, 1 - dz
                nc.sync.dma_start(out=Fm3[8 * s:8 * s + 8],
                                  in_=Fpad[8 * s:8 * s + 8, my:my + 48, mz:mz + 48])

            outbuf = epi.tile([6, YZ], F32, tag="outbuf")
            for r0, nr in ROW_CHUNKS:
                po = psf.tile([8, nr * 48], F32, tag="psf", name="po")
                nc.tensor.matmul(po, w["lhtSp"],
                                 Fpad[:, 1 + r0:1 + r0 + nr, 1:49],
                                 start=True, stop=False)
                nc.tensor.matmul(po, w["lhtSm"], Fm3[:, r0:r0 + nr, :],
                                 start=False, stop=True)
                nc.vector.tensor_add(out=outbuf[0:6, r0 * 48:(r0 + nr) * 48],
                                     in0=po[0:6, :],
                                     in1=qco[0:6, r0 * 48:(r0 + nr) * 48])
            nc.sync.dma_start(out=t["out0"][:].rearrange("p y z -> p (y z)"),
                              in_=outbuf)
    return t


_BUILT = {}


def _build(reps=1):
    if reps not in _BUILT:
        nc = bacc.Bacc()
        with tile.TileContext(nc) as tc:
            device_kernel(tc, reps=reps)
        nc.finalize()
        _BUILT[reps] = nc
    return _BUILT[reps]


def _host_constants(W0, b0, W1, b1, W2, b2, W3, b3, Wout, bout):
    import ml_dtypes
    BF = ml_dtypes.bfloat16
    kron = np.kron
    I8 = np.eye(8, dtype=np.float32)
    lhtA = np.zeros((16, 128), np.float32)
    lhtB = np.zeros((16, 128), np.float32)
    lhtAs = np.zeros((16, 128), np.float32)
    lhtBs = np.zeros((16, 128), np.float32)
    for p in range(8):
        for c in range(2):
            lhtA[2 * p + c, 16 * p:16 * p + 16] = W0[:, c]
            lhtB[2 * p + c, 16 * p:16 * p + 16] = W0[:, 2 + c]
    for p in range(7):
        for c in range(2):
            lhtAs[2 * (p + 1) + c, 16 * p:16 * p + 16] = W0[:, c]
            lhtBs[2 * (p + 1) + c, 16 * p:16 * p + 16] = W0[:, 2 + c]
    consts = {
        "lhtA": lhtA.astype(BF), "lhtB": lhtB.astype(BF),
        "lhtAs": lhtAs.astype(BF), "lhtBs": lhtBs.astype(BF),
        "lht1": kron(I8, W1.T).astype(BF),
        "lht2": kron(I8, W2.T).astype(BF),
        "lht3": kron(I8, W3.T).astype(BF),
    }
    op = kron(I8, Wout.T.reshape(16, 1)).astype(np.float32)
    consts["lhtOp"] = op.astype(BF)
    consts["lhtOm"] = (-op).astype(BF)
    for n, b in (("b0v", b0), ("b1v", b1), ("b2v", b2), ("b3v", b3)):
        consts[n] = np.tile(b, 8).reshape(128, 1).astype(np.float32)
    lhtSp = np.zeros((128, 8), np.float32)
    lhtSm = np.zeros((128, 8), np.float32)
    cvec = np.zeros((128, 1), np.float32)
    for s, (dx, dy, dz, dinv) in enumerate(SHIFTS_U):
        c = dinv * SCALE
        for b in range(8):
            cvec[8 * s + b, 0] = c
        for m in range(1, 7):
            lhtSp[8 * s + m, m - 1] = 1.0
            if dx == 1:
                lhtSm[8 * s + (m - 1), m - 1] = -1.0
            else:
                lhtSm[8 * s + m, m - 1] = -1.0
    consts["lhtSp"] = lhtSp.astype(BF)
    consts["lhtSm"] = lhtSm.astype(BF)
    consts["cvec"] = cvec
    return consts


def _make_in_maps(q, consts):
    qg = np.transpose(q[0], (3, 0, 1, 2))
    in_maps = []
    for c in range(N_CORES):
        planes = [(OWN * c - 1 + p) % NX for p in range(PLANES)]
        slab = np.transpose(qg[:, planes], (1, 0, 2, 3))
        qpad = np.pad(slab, [(0, 0), (0, 0), (1, 1), (1, 1)], mode="wrap")
        in_maps.append({"qpad": np.ascontiguousarray(qpad), **consts})
    return in_maps


def kernel(q, W0, b0, W1, b1, W2, b2, W3, b3, Wout, bout, _timing=None):
    q = np.asarray(q, np.float32)
    consts = _host_constants(W0, b0, W1, b1, W2, b2, W3, b3, Wout, bout)
    in_maps = _make_in_maps(q, consts)
    nc = _build()
    res = run_bass_kernel_spmd(nc, in_maps, core_ids=list(range(N_CORES)))
    out = np.array(q[0], copy=True)
    for c in range(N_CORES):
        out[OWN * c:OWN * c + OWN, :, :, 0] = res.results[c]["out0"]
    return out[None]


# revision 19
# speedup vs baseline: 1.2584x; 1.2584x over previous
"""Trainium2 Bass kernel for nn_AutomatonPT (3D cellular automaton / GNN message passing).

Full inputs -> full output. Shards the X axis across 8 NeuronCores (6 planes
each + 1 halo plane per side, periodic). Uses the antisymmetry of the pair
flux (F_ji = -F_ij) to evaluate only 13 of the 26 stencil shifts.

MLP chain in bf16 with fp32 PSUM accumulation; ACT reads PSUM; 4 independent
chains interleaved at chunk granularity with layer-0 tanh software-pipelined
one group ahead; epilogue in bf16 with exact +-1 scatter coefficients (the
dinv*SCALE factors are folded into the charge tensors); final add in fp32.
"""
import sys

sys.path.insert(0, "/opt/trn_rl_repo")
from contextlib import ExitStack

import numpy as np

import concourse.bass as bass
import concourse.bacc as bacc
import concourse.tile as tile
from concourse import mybir
from concourse.bass_utils import run_bass_kernel_spmd

F32 = mybir.dt.float32
BF16 = mybir.dt.bfloat16
ALU = mybir.AluOpType
ACTF = mybir.ActivationFunctionType

N_CORES = 8
NX = 48
PLANES = 8
OWN = 6
YZ = 48 * 48
PAD = 50 * 50

SCALE = 0.05234482976098482 * 0.8
S2 = 2 ** -0.5
S3 = 3 ** -0.5
# unordered pair deltas d = j - i: 9 with dx=1, then 4 with dx=0
SHIFTS_U = [
    (1, 0, 0, 1.0),
    (1, 1, 0, S2), (1, -1, 0, S2), (1, 0, 1, S2), (1, 0, -1, S2),
    (1, 1, 1, S3), (1, 1, -1, S3), (1, -1, 1, S3), (1, -1, -1, S3),
    (0, 1, 0, 1.0), (0, 0, 1, 1.0),
    (0, 1, 1, S2), (0, 1, -1, S2),
]

H_CHUNKS = [(0, 1536), (1536, 768)]
MM_N = 512
PSF_CHUNKS = [(0, 512), (512, 512), (1024, 512), (1536, 512), (2048, 256)]
ROW_CHUNKS = [(0, 10), (10, 10), (20, 10), (30, 10), (40, 8)]
GROUP = 2


def _v3(ap):
    return ap.rearrange("p (y z) -> p y z", y=48)


def device_kernel(tc, reps=1):
    nc = tc.nc
    t = {}
    t["qpad"] = nc.dram_tensor("qpad", [PLANES, 2, 50, 50], F32, kind="ExternalInput")
    for n in ("lhtA", "lhtB", "lhtAs", "lhtBs"):
        t[n] = nc.dram_tensor(n, [16, 128], BF16, kind="ExternalInput")
    for n in ("lht1", "lht2", "lht3"):
        t[n] = nc.dram_tensor(n, [128, 128], BF16, kind="ExternalInput")
    t["lhtOp"] = nc.dram_tensor("lhtOp", [128, 8], BF16, kind="ExternalInput")
    t["lhtOm"] = nc.dram_tensor("lhtOm", [128, 8], BF16, kind="ExternalInput")
    for n in ("b0v", "b1v", "b2v", "b3v"):
        t[n] = nc.dram_tensor(n, [128, 1], F32, kind="ExternalInput")
    t["lhtSp"] = nc.dram_tensor("lhtSp", [128, 8], BF16, kind="ExternalInput")
    t["lhtSm"] = nc.dram_tensor("lhtSm", [128, 8], BF16, kind="ExternalInput")
    t["cvec"] = nc.dram_tensor("cvec", [128, 1], F32, kind="ExternalInput")
    t["out0"] = nc.dram_tensor("out0", [OWN, 48, 48], F32, kind="ExternalOutput")

    with ExitStack() as ctx:
        persist = ctx.enter_context(tc.tile_pool(name="persist", bufs=1))
        mmps = ctx.enter_context(tc.tile_pool(name="mmps", bufs=2, space="PSUM"))
        psf = ctx.enter_context(tc.tile_pool(name="psf", bufs=2, space="PSUM"))

        w = {}
        wspecs = [("lhtA", [16, 128], BF16), ("lhtB", [16, 128], BF16),
                  ("lhtAs", [16, 128], BF16), ("lhtBs", [16, 128], BF16),
                  ("lht1", [128, 128], BF16), ("lht2", [128, 128], BF16),
                  ("lht3", [128, 128], BF16), ("lhtOp", [128, 8], BF16),
                  ("lhtOm", [128, 8], BF16), ("b0v", [128, 1], F32),
                  ("b1v", [128, 1], F32), ("b2v", [128, 1], F32),
                  ("b3v", [128, 1], F32), ("lhtSp", [128, 8], BF16),
                  ("lhtSm", [128, 8], BF16), ("cvec", [128, 1], F32)]
        for n, shape, dt in wspecs:
            w[n] = persist.tile(shape, dt, tag=n, name=n)
            nc.sync.dma_start(out=w[n], in_=t[n][:])

        fstack = persist.tile([128, YZ], BF16, tag="fstack")
        nc.vector.memset(fstack[96:128, :], 0.0)
        qc8 = persist.tile([8, 50, 50], F32, tag="qc8")
        nc.sync.dma_start(out=qc8, in_=t["qpad"][:, 0])
        # charge tensors in bf16, pre-scaled by dinv*SCALE (cvec), built early
        qc8b = persist.tile([8, 50, 50], BF16, tag="qc8b")
        nc.vector.tensor_copy(out=qc8b, in_=qc8)
        qcs8b = persist.tile([8, 50, 50], BF16, tag="qcs8b")
        nc.vector.memset(qcs8b[0:8], 0.0)
        nc.sync.dma_start(out=qcs8b[0:7], in_=qc8b[1:8])
        qo_rep = persist.tile([128, YZ], BF16, tag="qo")
        qn_rep = persist.tile([128, YZ], BF16, tag="qn")
        nc.vector.memset(qo_rep[96:128, :], 0.0)
        nc.vector.memset(qn_rep[96:128, :], 0.0)
        qo3, qn3 = _v3(qo_rep), _v3(qn_rep)
        for s, (dx, dy, dz, _) in enumerate(SHIFTS_U):
            ay, az = 1 + dy, 1 + dz
            nc.sync.dma_start(out=qo3[8 * s:8 * s + 8], in_=qc8b[:, 1:49, 1:49])
            qsrc = qcs8b if dx == 1 else qc8b
            nc.sync.dma_start(out=qn3[8 * s:8 * s + 8],
                              in_=qsrc[:, ay:ay + 48, az:az + 48])
        nc.vector.tensor_scalar_mul(out=qo_rep, in0=qo_rep, scalar1=w["cvec"])
        nc.vector.tensor_scalar_mul(out=qn_rep, in0=qn_rep, scalar1=w["cvec"])

        for _rep in range(reps):
          with tc.tile_pool(name=f"abfam{_rep}", bufs=1) as abfam:
            A8pad = abfam.tile([128, 50, 50], BF16, tag="A8pad")
            B8pad = abfam.tile([128, 50, 50], BF16, tag="B8pad")
            A8s = abfam.tile([128, 50, 50], BF16, tag="A8s")
            B8s = abfam.tile([128, 50, 50], BF16, tag="B8s")

            # ---- phase A: layer-0 features (fp32 matmul -> bf16) ----
            with tc.tile_pool(name=f"qpool{_rep}", bufs=1) as qpool:
                q16 = qpool.tile([16, PAD], F32, tag="q16")
                nc.sync.dma_start(out=q16,
                                  in_=t["qpad"][:].rearrange("p c y z -> (p c) (y z)"))
                q16b = qpool.tile([16, PAD], BF16, tag="q16b")
                nc.vector.tensor_copy(out=q16b, in_=q16)
                for dst, lht in ((A8pad, "lhtA"), (B8pad, "lhtB"),
                                 (A8s, "lhtAs"), (B8s, "lhtBs")):
                    dflat = dst.rearrange("p y z -> p (y z)")
                    for off in range(0, PAD, MM_N):
                        n = min(MM_N, PAD - off)
                        ps = mmps.tile([128, n], F32, tag="mm", name="mm")
                        nc.tensor.matmul(ps, w[lht], q16b[:, off:off + n],
                                         start=True, stop=True)
                        nc.scalar.copy(out=dflat[:, off:off + n], in_=ps)

            # ---- phase B: shifts x 2 directions, interleaved chains ----
            with tc.tile_pool(name=f"pre{_rep}", bufs=12) as prep, \
                 tc.tile_pool(name=f"hp{_rep}", bufs=12) as hp, \
                 tc.tile_pool(name=f"h3p{_rep}", bufs=3) as h3p, \
                 tc.tile_pool(name=f"fsp{_rep}", bufs=2) as fsp:

                def emit_pre(s):
                    dx, dy, dz, _ = SHIFTS_U[s]
                    f1pre = prep.tile([128, YZ], BF16, tag="pre", name="pre")
                    f2pre = prep.tile([128, YZ], BF16, tag="pre", name="pre")
                    ay, az = 1 + dy, 1 + dz
                    if dx == 1:
                        nc.vector.tensor_add(out=_v3(f1pre),
                                             in0=A8pad[:, 1:49, 1:49],
                                             in1=B8s[:, ay:ay + 48, az:az + 48])
                        nc.vector.tensor_add(out=_v3(f2pre),
                                             in0=A8s[:, ay:ay + 48, az:az + 48],
                                             in1=B8pad[:, 1:49, 1:49])
                    else:
                        nc.vector.tensor_add(out=_v3(f1pre),
                                             in0=A8pad[:, 1:49, 1:49],
                                             in1=B8pad[:, ay:ay + 48, az:az + 48])
                        nc.vector.tensor_add(out=_v3(f2pre),
                                             in0=A8pad[:, ay:ay + 48, az:az + 48],
                                             in1=B8pad[:, 1:49, 1:49])
                    return [f1pre, f2pre]

                def emit_h0s(pres):
                    h0s = [hp.tile([128, YZ], BF16, tag="h", name="h")
                           for _ in pres]
                    for h0, pre in zip(h0s, pres):
                        nc.scalar.activation(out=h0, in_=pre, func=ACTF.Tanh,
                                             bias=w["b0v"], scale=1.0)
                    return h0s

                LAYERS = [("lht1", "b1v"), ("lht2", "b2v"), ("lht3", "b3v")]

                def emit_group(shifts, chains):
                    for li, (lht, bv) in enumerate(LAYERS):
                        nxt = []
                        for ci in range(len(chains)):
                            if li == 2:
                                kt = "h3a" if ci % 2 == 0 else "h3b"
                                nxt.append(h3p.tile([128, YZ], BF16, tag=kt, name=kt))
                            else:
                                nxt.append(hp.tile([128, YZ], BF16, tag="h", name="h"))
                        for off, csz in H_CHUNKS:
                            pss = []
                            for ci, hcur in enumerate(chains):
                                ps = mmps.tile([128, csz], F32, tag="mm", name="mm")
                                for o2 in range(0, csz, MM_N):
                                    n2 = min(MM_N, csz - o2)
                                    nc.tensor.matmul(ps[:, o2:o2 + n2], w[lht],
                                                     hcur[:, off + o2:off + o2 + n2],
                                                     start=True, stop=True)
                                pss.append(ps)
                            for ci, ps in enumerate(pss):
                                nc.scalar.activation(out=nxt[ci][:, off:off + csz],
                                                     in_=ps, func=ACTF.Tanh,
                                                     bias=w[bv], scale=1.0)
                        chains = nxt
                    # tail: fdiff -> tanh -> fstack, interleaved across shifts
                    fss = {s: fsp.tile([8, YZ], BF16, tag="fs", name="fs")
                           for s in shifts}
                    for off, csz in PSF_CHUNKS:
                        pfs = {}
                        for gi, s in enumerate(shifts):
                            h3f1, h3f2 = chains[2 * gi], chains[2 * gi + 1]
                            pf = psf.tile([8, csz], F32, tag="psf", name="psf")
                            nc.tensor.matmul(pf, w["lhtOp"], h3f1[:, off:off + csz],
                                             start=True, stop=False)
                            nc.tensor.matmul(pf, w["lhtOm"], h3f2[:, off:off + csz],
                                             start=False, stop=True)
                            pfs[s] = pf
                        for s in shifts:
                            nc.scalar.activation(out=fss[s][:, off:off + csz],
                                                 in_=pfs[s], func=ACTF.Tanh)
                    for s in shifts:
                        nc.sync.dma_start(out=fstack[8 * s:8 * s + 8, :], in_=fss[s])

                groups = [list(range(i, min(i + GROUP, 13)))
                          for i in range(0, 13, GROUP)]
                # software pipeline: pre + h0 of group g+1 emitted before layers(g)
                h0bank = emit_h0s([p for s in groups[0] for p in emit_pre(s)])
                for g, shifts in enumerate(groups):
                    if g + 1 < len(groups):
                        h0_next = emit_h0s(
                            [p for s in groups[g + 1] for p in emit_pre(s)])
                    else:
                        h0_next = None
                    emit_group(shifts, h0bank)
                    h0bank = h0_next

          # ---- phase C: epilogue ----
          with tc.tile_pool(name=f"epi{_rep}", bufs=1) as epi:
            qco = epi.tile([6, YZ], F32, tag="qco")
            nc.sync.dma_start(out=_v3(qco), in_=qc8[1:7, 1:49, 1:49])
            Fq = epi.tile([128, YZ], BF16, tag="Fq")
            Fpad = epi.tile([128, 50, 50], BF16, tag="Fpad")
            nc.vector.scalar_tensor_tensor(out=Fq, in0=fstack, scalar=0.0,
                                           in1=qo_rep, op0=ALU.min, op1=ALU.mult)
            nc.vector.scalar_tensor_tensor(out=Fpad[:, 1:49, 1:49], in0=_v3(fstack),
                                           scalar=0.0, in1=qn3,
                                           op0=ALU.max, op1=ALU.mult)
            nc.vector.tensor_add(out=Fpad[:, 1:49, 1:49], in0=Fpad[:, 1:49, 1:49],
                                 in1=_v3(Fq))
            nc.sync.dma_start(out=Fpad[:, 1:49, 0:1], in_=Fpad[:, 1:49, 48:49])
            nc.sync.dma_start(out=Fpad[:, 1:49, 49:50], in_=Fpad[:, 1:49, 1:2])
            nc.sync.dma_start(out=Fpad[:, 0:1, 0:50], in_=Fpad[:, 48:49, 0:50])
            nc.sync.dma_start(out=Fpad[:, 49:50, 0:50], in_=Fpad[:, 1:2, 0:50])

            # pre-rolled F stack for the single merged minus-scatter matmul
            Fm = epi.tile([128, YZ], BF16, tag="Fm")
            Fm3 = _v3(Fm)
            for s, (dx, dy, dz, _) in enumerate(SHIFTS_U):
                my, mz = 1 - dy, 1 - dz
                nc.sync.dma_start(out=Fm3[8 * s:8 * s + 8],
                                  in_=Fpad[8 * s:8 * s + 8, my:my + 48, mz:mz + 48])

            outbuf = epi.tile([6, YZ], F32, tag="outbuf")
            for r0, nr in ROW_CHUNKS:
                po = psf.tile([8, nr * 48], F32, tag="psf", name="po")
                nc.tensor.matmul(po, w["lhtSp"],
                                 Fpad[:, 1 + r0:1 + r0 + nr, 1:49],
                                 start=True, stop=False)
                nc.tensor.matmul(po, w["lhtSm"], Fm3[:, r0:r0 + nr, :],
                                 start=False, stop=True)
                nc.vector.tensor_add(out=outbuf[0:6, r0 * 48:(r0 + nr) * 48],
                                     in0=po[0:6, :],
                                     in1=qco[0:6, r0 * 48:(r0 + nr) * 48])
            nc.sync.dma_start(out=t["out0"][:].rearrange("p y z -> p (y z)"),
                              in_=outbuf)
    return t


_BUILT = {}


def _build(reps=1):
    if reps not in _BUILT:
        nc = bacc.Bacc()
        with tile.TileContext(nc) as tc:
            device_kernel(tc, reps=reps)
        nc.finalize()
        _BUILT[reps] = nc
    return _BUILT[reps]


def _host_constants(W0, b0, W1, b1, W2, b2, W3, b3, Wout, bout):
    import ml_dtypes
    BF = ml_dtypes.bfloat16
    kron = np.kron
    I8 = np.eye(8, dtype=np.float32)
    lhtA = np.zeros((16, 128), np.float32)
    lhtB = np.zeros((16, 128), np.float32)
    lhtAs = np.zeros((16, 128), np.float32)
    lhtBs = np.zeros((16, 128), np.float32)
    for p in range(8):
        for c in range(2):
            lhtA[2 * p + c, 16 * p:16 * p + 16] = W0[:, c]
            lhtB[2 * p + c, 16 * p:16 * p + 16] = W0[:, 2 + c]
    for p in range(7):
        for c in range(2):
            lhtAs[2 * (p + 1) + c, 16 * p:16 * p + 16] = W0[:, c]
            lhtBs[2 * (p + 1) + c, 16 * p:16 * p + 16] = W0[:, 2 + c]
    consts = {
        "lhtA": lhtA.astype(BF), "lhtB": lhtB.astype(BF),
        "lhtAs": lhtAs.astype(BF), "lhtBs": lhtBs.astype(BF),
        "lht1": kron(I8, W1.T).astype(BF),
        "lht2": kron(I8, W2.T).astype(BF),
        "lht3": kron(I8, W3.T).astype(BF),
    }
    op = kron(I8, Wout.T.reshape(16, 1)).astype(np.float32)
    consts["lhtOp"] = op.astype(BF)
    consts["lhtOm"] = (-op).astype(BF)
    for n, b in (("b0v", b0), ("b1v", b1), ("b2v", b2), ("b3v", b3)):
        consts[n] = np.tile(b, 8).reshape(128, 1).astype(np.float32)
    lhtSp = np.zeros((128, 8), np.float32)
    lhtSm = np.zeros((128, 8), np.float32)
    cvec = np.zeros((128, 1), np.float32)
    for s, (dx, dy, dz, dinv) in enumerate(SHIFTS_U):
        c = dinv * SCALE
        for b in range(8):
            cvec[8 * s + b, 0] = c
        for m in range(1, 7):
            lhtSp[8 * s + m, m - 1] = 1.0
            if dx == 1:
                lhtSm[8 * s + (m - 1), m - 1] = -1.0
            else:
                lhtSm[8 * s + m, m - 1] = -1.0
    consts["lhtSp"] = lhtSp.astype(BF)
    consts["lhtSm"] = lhtSm.astype(BF)
    consts["cvec"] = cvec
    return consts


def _make_in_maps(q, consts):
    qg = np.transpose(q[0], (3, 0, 1, 2))
    in_maps = []
    for c in range(N_CORES):
        planes = [(OWN * c - 1 + p) % NX for p in range(PLANES)]
        slab = np.transpose(qg[:, planes], (1, 0, 2, 3))
        qpad = np.pad(slab, [(0, 0), (0, 0), (1, 1), (1, 1)], mode="wrap")
        in_maps.append({"qpad": np.ascontiguousarray(qpad), **consts})
    return in_maps


def kernel(q, W0, b0, W1, b1, W2, b2, W3, b3, Wout, bout, _timing=None):
    q = np.asarray(q, np.float32)
    consts = _host_constants(W0, b0, W1, b1, W2, b2, W3, b3, Wout, bout)
    in_maps = _make_in_maps(q, consts)
    nc = _build()
    res = run_bass_kernel_spmd(nc, in_maps, core_ids=list(range(N_CORES)))
    out = np.array(q[0], copy=True)
    for c in range(N_CORES):
        out[OWN * c:OWN * c + OWN, :, :, 0] = res.results[c]["out0"]
    return out[None]
